# revision 28
# baseline (speedup 1.0000x reference)
"""NT-Xent loss kernel for Trainium2, 8 NeuronCores, Bass/Tile.

Contract: kernel(zi, zj) takes FULL inputs (4096, 128) f32 each and returns
the FULL scalar loss (np.float32), matching:

    z   = concat(zi, zj)                       # (8192, 128)
    zn  = z / max(||z||, 1e-8)
    sim = zn @ zn.T
    lse_i  = log(sum_{j != i} exp(sim_ij / T))
    pos_i  = sim[i, (i + 4096) % 8192] / T
    loss   = mean(lse - pos)                   # T = 0.5

Sharding: data-parallel over rows with symmetry exploitation, no
collectives. Core k receives the normalized, transposed representations
znT in bf16 with columns rolled by -1024*k, so its 1024 local rows are
columns [0, 1024). In rolled coordinates every core runs the identical
program over column blocks j (block j = columns [1024j, 1024(j+1)),
i.e. global rows 1024(k+j) mod 8192):

  j = 0      : diagonal block, direct, row-sums only (self-diag extracted)
  j = 1,2,3  : direct row-sums + column-sums (the mirrored entries for the
               rows of core k+j) accumulated on PE via identity-matmuls
  j = 4      : direct row-sums on both cores of the pair (no symmetry;
               positive-pair diagonal extracted here)

Columns [5120, 8192) of each core's view are exactly the mirrors of
blocks j=1,2,3 computed by cores k-5..k-7 (mod 8); the host merges
row-sums, column-sums, self/positive diagonals and finishes with
log/subtract/mean in float64 (~7k values per core).
"""

import os
import sys

import numpy as np

for _p in ("/opt/trn_rl_repo", "/root/.axon_site/_ro/trn_rl_repo"):
    if os.path.isdir(_p) and _p not in sys.path:
        sys.path.append(_p)

import ml_dtypes  # noqa: E402

import concourse.bass as bass  # noqa: E402,F401
import concourse.tile as tile  # noqa: E402
from concourse import bacc, mybir  # noqa: E402
from concourse.bass_utils import run_bass_kernel_spmd  # noqa: E402

B = 4096
D = 128
N2 = 2 * B               # 8192 rows total
NCORES = 8
LOCAL = N2 // NCORES     # 1024 rows per core
P = 128                  # partitions
MCHUNK = LOCAL // P      # 8 local row chunks
BLK = 1024               # column block = one core's rows
NBLK = 5                 # blocks computed per core (j = 0..4)
NCOLS = NBLK * BLK       # 5120 columns of znT actually used
NTILE = 512              # matmul moving free dim (1 PSUM bank)
CS_BLOCKS = (1, 2, 3)    # blocks with column-sum accumulation
INV_T = 2.0              # 1 / TEMP
EPS = 1e-8               # reference norm clamp

F32 = mybir.dt.float32
BF16 = mybir.dt.bfloat16
AF = mybir.ActivationFunctionType
ALU = mybir.AluOpType


def build_program(reps: int = 1, no_colsum: bool = False,
                  es_f32: bool = False):
    """Build + compile the per-core Bass program (identical on all cores).
    reps > 1 repeats the compute body (same outputs) for timing calibration:
    (T(R) - T(1)) / (R - 1) isolates the steady-state kernel time from
    dispatch overhead.

    Column blocks are processed as three groups per row-chunk, sized to
    the 8 PSUM banks (2 tiles in flight):
      G0 = blocks {0, 1} (2048 cols; block 1 needs column sums)
      G1 = blocks {2, 3} (2048 cols; both need column sums)
      G2 = block 4 (1024 cols; computed on both cores of the pair)
    Row sums ride on the ScalarE exp pass (accum_out). Column sums are
    reduced to 4 m-pair partial tiles per group on VectorE (bf16 2x adds)
    and shipped to the host, which finishes the partition reduction.
    """
    nc = bacc.Bacc("TRN2", target_bir_lowering=False, debug=False,
                   num_devices=NCORES)
    znt_ap = nc.dram_tensor("znt", [P, NCOLS], BF16,
                            kind="ExternalInput").ap()
    # per-row exp row-sums (group, m); self/positive diagonals are
    # recomputed on the host from the same bf16 znt
    out_ap = nc.dram_tensor("out", [P, 24], F32, kind="ExternalOutput").ap()
    # exp column partial sums: 4 m-pair tiles of [block1 | blocks 2+3]
    cols_ap = nc.dram_tensor("cols", [P, 4 * 3072], BF16,
                             kind="ExternalOutput").ap()

    # (column offset, width, cs offset within group, cs width, cols slot)
    groups = ((0, 2 * BLK, BLK, BLK, 0), (2 * BLK, 2 * BLK, 0, 2 * BLK, BLK),
              (4 * BLK, BLK, 0, 0, None))

    with tile.TileContext(nc) as tc:
        with (
            tc.tile_pool(name="persist", bufs=1) as persist,
            tc.tile_pool(name="espool", bufs=4) as espool,
            tc.tile_pool(name="pairpool", bufs=2) as pairpool,
        ):
            znt = persist.tile([P, NCOLS], BF16)
            stats = persist.tile([P, 24], F32)
            sums = stats[:, 0:24]
            warm = persist.tile([P, 1], F32)

            # preload the exp activation table while DMAs run
            nc.vector.memset(warm[:], 0.0)
            nc.scalar.activation(warm[:], warm[:], AF.Exp, scale=INV_T)

            # the first group needs columns [0, 2048): load them in
            # half-block chunks across both DMA issue queues
            nc.gpsimd.dma_start(out=znt[:, 512:BLK], in_=znt_ap[:, 512:BLK])
            nc.sync.dma_start(out=znt[:, 0:512], in_=znt_ap[:, 0:512])
            nc.sync.dma_start(out=znt[:, BLK:BLK + 512],
                              in_=znt_ap[:, BLK:BLK + 512])
            nc.gpsimd.dma_start(out=znt[:, BLK + 512:2 * BLK],
                                in_=znt_ap[:, BLK + 512:2 * BLK])
            for h, eng in ((2, nc.sync), (3, nc.gpsimd), (4, nc.sync)):
                eng.dma_start(out=znt[:, h * BLK:(h + 1) * BLK],
                              in_=znt_ap[:, h * BLK:(h + 1) * BLK])

            with tc.tile_pool(name="psum1", bufs=2, space="PSUM") as psum1:
                for rep in range(reps):
                    for g, (lo, width, cslo, csw, slot) in enumerate(groups):
                        do_cs = csw > 0 and not no_colsum
                        es_tiles = {}
                        for m in range(MCHUNK):
                            lhs = znt[:, m * P:(m + 1) * P]
                            pt = psum1.tile([P, width], F32, tag="pt",
                                            padded_shape=[P, 2 * BLK])
                            for h in range(width // NTILE):
                                nc.tensor.matmul(
                                    pt[:, h * NTILE:(h + 1) * NTILE],
                                    lhsT=lhs,
                                    rhs=znt[:, lo + h * NTILE:
                                            lo + (h + 1) * NTILE],
                                    start=True, stop=True)
                            es = espool.tile([P, width],
                                             F32 if es_f32 else BF16,
                                             tag="es",
                                             padded_shape=[P, 2 * BLK])
                            c = g * MCHUNK + m
                            nc.scalar.activation(es[:], pt[:], AF.Exp,
                                                 scale=INV_T,
                                                 accum_out=sums[:, c:c + 1])
                            es_tiles[m] = es
                            if do_cs and m % 2 == 1:
                                # column partial sums: one VectorE add per
                                # m-pair (bf16 2x), shipped to the host
                                pr = pairpool.tile(
                                    [P, csw], BF16, tag=f"pr{g}",
                                    name=f"pr{g}")
                                ea = es_tiles.pop(m - 1)
                                eb = es_tiles.pop(m)
                                nc.vector.tensor_add(
                                    pr[:], ea[:, cslo:cslo + csw],
                                    eb[:, cslo:cslo + csw])
                                q = m // 2
                                o = q * 3072 + slot
                                nc.sync.dma_start(
                                    out=cols_ap[:, o:o + csw], in_=pr[:])

            nc.sync.dma_start(out=out_ap[:], in_=stats[:])

    nc.compile()
    return nc


_STATE: dict = {}


def _get_program(reps: int = 1, **kw):
    key = f"nc{reps}{sorted(kw.items())}"
    if key not in _STATE:
        _STATE[key] = build_program(reps, **kw)
    return _STATE[key]


def make_in_maps(z: np.ndarray) -> tuple[list[dict], np.ndarray]:
    """Host prep: normalize rows (fp32, matching reference), cast bf16,
    transpose, and roll columns per core (keeping the used 5120 columns).
    Returns (per-core input maps, full znt [128, 8192])."""
    norm = np.sqrt(np.einsum("ij,ij->i", z, z, dtype=np.float32,
                             optimize=True))
    norm = np.maximum(norm, np.float32(EPS))
    zn = z / norm[:, None]
    znt = np.ascontiguousarray(zn.astype(ml_dtypes.bfloat16).T)  # [128, 8192]
    zntw = np.concatenate([znt, znt[:, :NCOLS]], axis=1)  # wraparound view
    in_maps = []
    for k in range(NCORES):
        s = k * LOCAL
        in_maps.append({"znt": np.ascontiguousarray(zntw[:, s:s + NCOLS])})
    return in_maps, znt


def host_rows(outs: list[np.ndarray], cols: list[np.ndarray],
              znt: np.ndarray) -> np.ndarray:
    """outs[k] = [128, 40] row-sums, cols[k] = [128, 3072] bf16 column
    partial sums, znt = normalized bf16 representations transposed
    [128, 8192]; returns per-row lse - pos/T (float64)."""
    S = np.zeros(N2, dtype=np.float64)
    for k in range(NCORES):
        o = outs[k].astype(np.float64)
        # row-sums: sums[p, g*8+m] -> local row 128m+p, sum over groups
        rs = o.reshape(P, 3, MCHUNK).sum(axis=1)               # [p, m]
        sl = slice(k * LOCAL, (k + 1) * LOCAL)
        S[sl] += rs.transpose(1, 0).reshape(-1)
        # column-sums: pair tiles [128, 4, 3072] = [block1 | blocks 2+3];
        # block j covers global rows 1024(k+j) mod 8192
        c = cols[k].astype(np.float32).reshape(P, 4, 3072)
        csum = c.sum(axis=(0, 1), dtype=np.float64)            # [3072]
        for ci, j in enumerate(CS_BLOCKS):
            tgt = ((k + j) % NCORES) * LOCAL
            S[tgt:tgt + LOCAL] += csum[ci * BLK:(ci + 1) * BLK]
    # self/positive diagonals from the same bf16 representations the
    # device used (fp32 dot products, matching the PE up to summation
    # order, ~1e-7 relative)
    zf = znt.astype(np.float32)
    self_full = np.einsum("di,di->i", zf, zf, dtype=np.float64)
    posm = np.roll(zf, -B, axis=1)
    pos2_full = INV_T * np.einsum("di,di->i", zf, posm, dtype=np.float64)
    S -= np.exp(INV_T * self_full)
    return np.log(S) - pos2_full


def host_finalize(outs: list[np.ndarray], cols: list[np.ndarray],
                  znt: np.ndarray) -> np.float32:
    return np.float32(host_rows(outs, cols, znt).mean())


def kernel(zi: np.ndarray, zj: np.ndarray) -> np.ndarray:
    zi = np.asarray(zi, dtype=np.float32)
    zj = np.asarray(zj, dtype=np.float32)
    assert zi.shape == (B, D) and zj.shape == (B, D), (zi.shape, zj.shape)
    z = np.concatenate([zi, zj], axis=0)

    nc = _get_program()
    in_maps, znt = make_in_maps(z)
    res = run_bass_kernel_spmd(nc, in_maps, list(range(NCORES)))
    return host_finalize([res.results[k]["out"] for k in range(NCORES)],
                         [res.results[k]["cols"] for k in range(NCORES)],
                         znt)


if __name__ == "__main__":
    rng = np.random.default_rng(0)
    zi = rng.standard_normal((B, D), dtype=np.float32)
    zj = rng.standard_normal((B, D), dtype=np.float32)
    print("loss:", kernel(zi, zj))
